# revision 58
# baseline (speedup 1.0000x reference)
"""Fused LyapunovThinkingBlock kernel for 8x TRN2 NeuronCores.

Math (B=32768, D=896): the reference block is
    q,k unused: softmax over a length-1 axis is exactly 1.0 => ctx == v
    v     = phi_x @ Wv^T + b_v
    h_att = v @ Wo^T + b_o
    g1    = silu([h_t, h_att] @ w1^T + b1)
    g2    = g1 @ w2^T + b2
    out   = h_t + LN(g2) * ln_g + ln_b

Weight folding (host, fp64):
    h_att = phi_x @ (Wo Wv)^T + (Wo b_v + b_o)
    [h_t, h_att] @ w1^T = h_t @ W1a^T + h_att @ W1b^T   (w1 = [W1a | W1b])
    => g1 = silu(h_t @ W1a^T + phi_x @ Bf^T + c)
       Bf = W1b Wo Wv,  c = b1 + W1b (Wo b_v + b_o)

Device (pure data parallel, batch sharded over 8 cores, 4096 rows each):
    stage 1 (feature-major): y1T[d,r] accumulated over 14 K-chunk matmuls,
        silu+bias on ScalarE directly from PSUM -> g1T (bf16) in SBUF
    stage 2 (row-major): y2[r,d] = g1 @ w2^T via activation-as-stationary
    stage 3: LayerNorm (bn_stats on DVE, fast-rsqrt Newton chain on Pool,
        normalize on ScalarE, residual add on Pool), store.

All matmul operands are bf16 (1 cycle/row, same as fp32r, half the DMA).
DMA queues: SP carries the critical weight+activation load stream with
per-K-chunk granularity (first matmul starts ~1.5us in); the Activation
queue carries h_t row-major prefetches and output stores.
"""

import numpy as np
import ml_dtypes

import concourse.bacc as bacc
import concourse.bass as bass
import concourse.mybir as mybir
import concourse.tile as tile
from concourse.bass_utils import run_bass_kernel_spmd

B, D = 32768, 896
N_CORES = 8
RPC = B // N_CORES            # rows per core = 4096
P = 128
KC = D // P                   # 7 K-chunks of 128
BLK = 512                     # rows per block
NBLK = RPC // BLK             # 8
BR = BLK // P                 # row-tiles per block = 4
NH = 448                      # stage-2 N chunk (2x448 = 896)
LN_EPS = 1e-5
RSQRT_MAGIC = 0x5F375A86      # fast inverse sqrt seed constant

F32 = mybir.dt.float32
BF16 = mybir.dt.bfloat16
I32 = mybir.dt.int32

BF = ml_dtypes.bfloat16

# test.py can flip these before calling kernel()
TRACE = False
_last_results = None


def _bcast_ap(ap, parts=P):
    return bass.AP(tensor=ap.tensor, offset=ap.offset, ap=[[0, parts], *ap.ap])


def _build(b2_zero: bool, ln_trivial: bool):
    nc = bacc.Bacc(None, target_bir_lowering=False)

    htT = nc.dram_tensor("htT", [D, RPC], BF16, kind="ExternalInput")
    pxT = nc.dram_tensor("pxT", [D, RPC], BF16, kind="ExternalInput")
    ht_row = nc.dram_tensor("ht_row", [RPC, D], BF16, kind="ExternalInput")
    AT_d = nc.dram_tensor("AT", [D, D], BF16, kind="ExternalInput")
    BfT_d = nc.dram_tensor("BfT", [D, D], BF16, kind="ExternalInput")
    w2T_d = nc.dram_tensor("w2T", [D, D], BF16, kind="ExternalInput")
    c_d = nc.dram_tensor("c_t", [P, KC], F32, kind="ExternalInput")
    if not b2_zero:
        b2_d = nc.dram_tensor("b2", [D], F32, kind="ExternalInput")
    if not ln_trivial:
        lng_d = nc.dram_tensor("ln_g", [D], F32, kind="ExternalInput")
        lnb_d = nc.dram_tensor("ln_b", [D], F32, kind="ExternalInput")
    out_d = nc.dram_tensor("out", [RPC, D], F32, kind="ExternalOutput")

    with tile.TileContext(nc) as tc:
        with (
            tc.tile_pool(name="wpool", bufs=1) as wpool,
            tc.tile_pool(name="xpool", bufs=4) as xpool,
            tc.tile_pool(name="gpool", bufs=2) as gpool,
            tc.tile_pool(name="spool", bufs=8) as spool,
            tc.tile_pool(name="hpool", bufs=2) as hpool,
            tc.tile_pool(name="opool", bufs=6) as opool,
            tc.tile_pool(name="ps1", bufs=2, space="PSUM") as ps1p,
            tc.tile_pool(name="ps2a", bufs=3, space="PSUM") as ps2ap,
            tc.tile_pool(name="ps2b", bufs=3, space="PSUM") as ps2bp,
        ):
            # ---- persistent weights ----
            wA = wpool.tile([P, KC, D], BF16)
            wB = wpool.tile([P, KC, D], BF16)
            w2 = wpool.tile([P, KC, D], BF16)
            AT_v = AT_d.rearrange("(kc p) n -> p kc n", p=P)
            BfT_v = BfT_d.rearrange("(kc p) n -> p kc n", p=P)
            w2T_v = w2T_d.rearrange("(kc p) n -> p kc n", p=P)
            cT = wpool.tile([P, KC], F32)
            magic_t = wpool.tile([P, 1], I32)
            nc.vector.memset(magic_t[:], RSQRT_MAGIC)
            if not b2_zero:
                b2b = wpool.tile([P, D], F32)
                nc.gpsimd.dma_start(out=b2b[:], in_=_bcast_ap(b2_d[:]))
            if not ln_trivial:
                lngb = wpool.tile([P, D], F32)
                nc.gpsimd.dma_start(out=lngb[:], in_=_bcast_ap(lng_d[:]))
                lnbb = wpool.tile([P, D], F32)
                nc.gpsimd.dma_start(out=lnbb[:], in_=_bcast_ap(lnb_d[:]))

            htT_v = htT.rearrange("(kc p) n -> p kc n", p=P)
            pxT_v = pxT.rearrange("(kc p) n -> p kc n", p=P)
            htR_v = ht_row.rearrange("(nb br p) d -> nb p br d", br=BR, p=P)

            def emit_loads(blk, xh, xp, htr):
                cs = slice(blk * BLK, (blk + 1) * BLK)
                if blk == 0:
                    # prologue split across both HWDGE queues (each has
                    # ~0.6us/DMA descriptor cost): weights on SP, activation
                    # chunks on the Act queue (no Act compute queued yet, so
                    # no sequencer head-of-line blocking). The first matmul
                    # needs only wA[:,0,0:128] + xh[:,0] — land those first.
                    nc.sync.dma_start(out=wA[:, 0, 0:P], in_=AT_v[:, 0, 0:P])
                    nc.scalar.dma_start(out=xh[:, 0], in_=htT_v[:, 0, cs])
                    nc.sync.dma_start(out=wA[:, 0, P:D], in_=AT_v[:, 0, P:D])
                    for k in range(1, KC):
                        nc.sync.dma_start(out=wA[:, k], in_=AT_v[:, k])
                        nc.scalar.dma_start(out=xh[:, k], in_=htT_v[:, k, cs])
                    for k in range(KC):
                        nc.sync.dma_start(out=wB[:, k], in_=BfT_v[:, k])
                        nc.scalar.dma_start(out=xp[:, k], in_=pxT_v[:, k, cs])
                    # cT is first needed by the silu burst at the end of
                    # block 0's stage 1 — keep it off the critical path;
                    # htr(b0) is issued later by the driver (needed ~30us in)
                    nc.scalar.dma_start(out=cT[:], in_=c_d[:])
                else:
                    # steady state: bulk loads on SP only (fewer, bigger
                    # transfers; Act queue must stay clear of compute deps)
                    nc.sync.dma_start(out=xh[:], in_=htT_v[:, :, cs])
                    nc.sync.dma_start(out=xp[:], in_=pxT_v[:, :, cs])
                    nc.sync.dma_start(out=htr[:], in_=htR_v[blk])

            # ---- stage 1: y1T = A^T-chunks . htT + Bf^T-chunks . pxT ----
            def emit_stage1(blk, xh, xp, g1):
                if blk == 0:
                    # k-outer with all 7 m-chain PSUM banks open: PE does 7
                    # matmuls per arriving weight chunk, pacing the cold
                    # start at DMA speed instead of waiting for all chunks
                    # borrow one bank-sized slot per chain across the pools
                    pools = [ps1p, ps1p, ps2ap, ps2ap, ps2ap, ps2bp, ps2bp]
                    tags = ["ps1", "ps1", "ps2a", "ps2a", "ps2a", "ps2b", "ps2b"]
                    ps1s = [pools[m].tile([P, BLK], F32, tag=tags[m],
                                          name=f"ps1k_{m}") for m in range(KC)]
                    for k in range(KC):
                        for m in range(KC):
                            nc.tensor.matmul(ps1s[m][:], wA[:, k, m * P:(m + 1) * P],
                                             xh[:, k], start=(k == 0), stop=False)
                    for k in range(KC):
                        for m in range(KC):
                            nc.tensor.matmul(ps1s[m][:], wB[:, k, m * P:(m + 1) * P],
                                             xp[:, k], start=False,
                                             stop=(k == KC - 1))
                            if k == KC - 1:
                                nc.scalar.activation(
                                    g1[:, m], ps1s[m][:],
                                    mybir.ActivationFunctionType.Silu,
                                    bias=cT[:, m:m + 1], scale=1.0)
                else:
                    for m in range(KC):
                        ms = slice(m * P, (m + 1) * P)
                        ps1 = ps1p.tile([P, BLK], F32, tag="ps1")
                        for k in range(KC):
                            nc.tensor.matmul(ps1[:], wA[:, k, ms], xh[:, k],
                                             start=(k == 0), stop=False)
                        for k in range(KC):
                            nc.tensor.matmul(ps1[:], wB[:, k, ms], xp[:, k],
                                             start=False, stop=(k == KC - 1))
                        # g1 = silu(y1 + c), evicted by ScalarE, feature-major
                        nc.scalar.activation(g1[:, m], ps1[:],
                                             mybir.ActivationFunctionType.Silu,
                                             bias=cT[:, m:m + 1], scale=1.0)

            # ---- stage 2 + 3 per 128-row tile ----
            def emit_stage23(blk, g1, htr):
                for r in range(BR):
                    rows = slice(blk * BLK + r * P, blk * BLK + (r + 1) * P)
                    rs = slice(r * P, (r + 1) * P)
                    ps2a = ps2ap.tile([P, NH], F32, tag="ps2a")
                    ps2b = ps2bp.tile([P, NH], F32, tag="ps2b")
                    for k in range(KC):
                        nc.tensor.matmul(ps2a[:], g1[:, k, rs], w2[:, k, 0:NH],
                                         start=(k == 0), stop=(k == KC - 1))
                    for k in range(KC):
                        nc.tensor.matmul(ps2b[:], g1[:, k, rs], w2[:, k, NH:D],
                                         start=(k == 0), stop=(k == KC - 1))

                    if b2_zero:
                        y0, y1 = ps2a[:], ps2b[:]
                    else:
                        yb = opool.tile([P, D], F32, tag="yb")
                        nc.vector.tensor_add(yb[:, 0:NH], ps2a[:], b2b[:, 0:NH])
                        nc.vector.tensor_add(yb[:, NH:D], ps2b[:], b2b[:, NH:D])
                        y0, y1 = yb[:, 0:NH], yb[:, NH:D]

                    # LN stats on DVE
                    stats = spool.tile([P, 2, 6], F32, tag="stats")
                    nc.vector.bn_stats(out=stats[:, 0], in_=y0)
                    nc.vector.bn_stats(out=stats[:, 1], in_=y1)
                    mv = spool.tile([P, 2], F32, tag="mv")
                    nc.vector.bn_aggr(out=mv[:], in_=stats[:])

                    # rstd = 1/sqrt(var+eps): fast-inverse-sqrt seed + 2
                    # Newton iterations, on DVE (cheap small-op overhead)
                    t0 = spool.tile([P, 1], F32, tag="t0")
                    nc.vector.tensor_scalar(t0[:], mv[:, 1:2], LN_EPS, None,
                                            mybir.AluOpType.add)
                    t1 = spool.tile([P, 1], I32, tag="t1")
                    nc.vector.tensor_scalar(t1[:], t0.bitcast(I32)[:], 1, None,
                                            mybir.AluOpType.logical_shift_right)
                    yr = spool.tile([P, 1], F32, tag="yr")
                    nc.vector.tensor_sub(yr.bitcast(I32)[:], magic_t[:], t1[:])
                    for _ in range(2):
                        a = spool.tile([P, 1], F32, tag="nt")
                        nc.vector.tensor_mul(a[:], yr[:], yr[:])
                        nc.vector.tensor_mul(a[:], a[:], t0[:])
                        nc.vector.tensor_scalar(a[:], a[:], -0.5, 1.5,
                                                mybir.AluOpType.mult,
                                                mybir.AluOpType.add)
                        nc.vector.tensor_mul(yr[:], yr[:], a[:])
                    nmr = spool.tile([P, 1], F32, tag="nmr")
                    nc.vector.scalar_tensor_tensor(
                        out=nmr[:], in0=mv[:, 0:1], scalar=-1.0, in1=yr[:],
                        op0=mybir.AluOpType.mult, op1=mybir.AluOpType.mult)

                    # normalize: half0 on ScalarE (Identity: in*rstd + nmr),
                    # half1 on DVE (tensor_scalar) — parallel engine paths.
                    # The very last row-tile does both halves on DVE: the
                    # rsqrt chain lives there, so no cross-engine sem hop
                    # and no queueing behind rt2's Act work in the drain.
                    o = opool.tile([P, D], F32, tag="o")
                    nc.scalar.activation(o[:, 0:NH], y0,
                                         mybir.ActivationFunctionType.Identity,
                                         bias=nmr[:], scale=yr[:])
                    nc.vector.tensor_scalar(o[:, NH:D], y1, yr[:], nmr[:],
                                            mybir.AluOpType.mult,
                                            mybir.AluOpType.add)
                    if not ln_trivial:
                        nc.vector.tensor_mul(o[:], o[:], lngb[:])
                    # residual adds on Pool (keeps DVE free for the next
                    # row-tile's stats/rsqrt chain); the very last row-tile
                    # adds on DVE so the drain doesn't queue behind Pool
                    if blk == NBLK - 1 and r == BR - 1:
                        nc.vector.tensor_add(o[:, 0:NH], o[:, 0:NH], htr[:, r, 0:NH])
                        nc.vector.tensor_add(o[:, NH:D], o[:, NH:D], htr[:, r, NH:D])
                    else:
                        nc.gpsimd.tensor_add(o[:, 0:NH], o[:, 0:NH], htr[:, r, 0:NH])
                        nc.gpsimd.tensor_add(o[:, NH:D], o[:, NH:D], htr[:, r, NH:D])
                    if not ln_trivial:
                        nc.vector.tensor_add(o[:], o[:], lnbb[:])
                    if blk == NBLK - 1:
                        # last block: store halves as they complete
                        nc.sync.dma_start(out=out_d[rows, 0:NH], in_=o[:, 0:NH])
                        nc.sync.dma_start(out=out_d[rows, NH:D], in_=o[:, NH:D])
                    else:
                        nc.sync.dma_start(out=out_d[rows, :], in_=o[:])

            # block-level software pipeline: emit s1(b) before s2(b-1) so
            # the in-order PE stream always has independent matmul work
            # while the Act engine drains a block's silu evictions; loads
            # prefetch one block ahead, w2 after block-1's loads (first
            # needed at s2(b0), which runs after s1(b1))
            tiles = {}

            def alloc_and_load(blk):
                xh = xpool.tile([P, KC, BLK], BF16, name="xh")
                xp = xpool.tile([P, KC, BLK], BF16, name="xp")
                htr = hpool.tile([P, BR, D], BF16, name="htr")
                tiles[blk] = (xh, xp, htr)
                emit_loads(blk, xh, xp, htr)

            def run_stage1(blk):
                g1 = gpool.tile([P, KC, BLK], BF16, name="g1")
                tiles[blk] = (*tiles[blk], g1)
                emit_stage1(blk, tiles[blk][0], tiles[blk][1], g1)

            alloc_and_load(0)
            alloc_and_load(1)
            nc.sync.dma_start(out=tiles[0][2][:], in_=htR_v[0])
            for k in range(KC):
                nc.sync.dma_start(out=w2[:, k], in_=w2T_v[:, k])
            run_stage1(0)
            for blk in range(1, NBLK):
                if blk + 1 < NBLK:
                    alloc_and_load(blk + 1)
                run_stage1(blk)
                emit_stage23(blk - 1, tiles[blk - 1][3], tiles[blk - 1][2])
            emit_stage23(NBLK - 1, tiles[NBLK - 1][3], tiles[NBLK - 1][2])

    nc.compile()
    return nc


def prepare(h_t, phi_x, in_proj_w, in_proj_b, out_proj_w, out_proj_b,
            w1, b1, w2, b2, ln_g, ln_b):
    """Host-side folding + build; returns (nc, in_maps)."""
    # ---- host-side weight folding (fp64) ----
    Wv = in_proj_w[2 * D:].astype(np.float64)
    bv = in_proj_b[2 * D:].astype(np.float64)
    Wo = out_proj_w.astype(np.float64)
    W1a = w1[:, :D].astype(np.float64)
    W1b = w1[:, D:].astype(np.float64)
    WoWv = Wo @ Wv
    Bf = W1b @ WoWv
    c = b1.astype(np.float64) + W1b @ (Wo @ bv + out_proj_b.astype(np.float64))

    AT = np.ascontiguousarray(W1a.T).astype(BF)
    BfT = np.ascontiguousarray(Bf.T).astype(BF)
    w2T = np.ascontiguousarray(np.asarray(w2, np.float64).T).astype(BF)
    c_t = np.ascontiguousarray(c.reshape(KC, P).T).astype(np.float32)

    b2_zero = bool(np.all(b2 == 0))
    ln_trivial = bool(np.all(ln_g == 1) and np.all(ln_b == 0))

    nc = _build(b2_zero, ln_trivial)

    h_t = np.asarray(h_t, dtype=np.float32)
    phi_x = np.asarray(phi_x, dtype=np.float32)

    in_maps = []
    for i in range(N_CORES):
        rows = slice(i * RPC, (i + 1) * RPC)
        ht_i = h_t[rows]
        px_i = phi_x[rows]
        m = {
            "htT": np.ascontiguousarray(ht_i.T).astype(BF),
            "pxT": np.ascontiguousarray(px_i.T).astype(BF),
            "ht_row": ht_i.astype(BF),
            "AT": AT,
            "BfT": BfT,
            "w2T": w2T,
            "c_t": c_t,
        }
        if not b2_zero:
            m["b2"] = np.asarray(b2, dtype=np.float32)
        if not ln_trivial:
            m["ln_g"] = np.asarray(ln_g, dtype=np.float32)
            m["ln_b"] = np.asarray(ln_b, dtype=np.float32)
        in_maps.append(m)

    return nc, in_maps


def kernel(**inputs):
    global _last_results
    nc, in_maps = prepare(**inputs)
    res = run_bass_kernel_spmd(nc, in_maps, core_ids=list(range(N_CORES)),
                               trace=TRACE)
    _last_results = res
    return np.concatenate([r["out"] for r in res.results], axis=0)


# revision 72
# speedup vs baseline: 1.2117x; 1.2117x over previous
"""Fused LyapunovThinkingBlock kernel for 8x TRN2 NeuronCores.

Math (B=32768, D=896): the reference block is
    q,k unused: softmax over a length-1 axis is exactly 1.0 => ctx == v
    v     = phi_x @ Wv^T + b_v
    h_att = v @ Wo^T + b_o
    g1    = silu([h_t, h_att] @ w1^T + b1)
    g2    = g1 @ w2^T + b2
    out   = h_t + LN(g2) * ln_g + ln_b

Weight folding (host, fp64):
    h_att = phi_x @ (Wo Wv)^T + (Wo b_v + b_o)
    [h_t, h_att] @ w1^T = h_t @ W1a^T + h_att @ W1b^T   (w1 = [W1a | W1b])
    => g1 = silu(h_t @ W1a^T + phi_x @ Bf^T + c)
       Bf = W1b Wo Wv,  c = b1 + W1b (Wo b_v + b_o)

Device (pure data parallel, batch sharded over 8 cores, 4096 rows each):
    stage 1 (feature-major): y1T[d,r] accumulated over 14 K-chunk matmuls,
        silu+bias on ScalarE directly from PSUM -> g1T (bf16) in SBUF
    stage 2 (row-major): y2[r,d] = g1 @ w2^T via activation-as-stationary
    stage 3: LayerNorm (bn_stats on DVE, fast-rsqrt Newton chain on Pool,
        normalize on ScalarE, residual add on Pool), store.

All matmul operands are bf16 (1 cycle/row, same as fp32r, half the DMA).
DMA queues: SP carries the critical weight+activation load stream with
per-K-chunk granularity (first matmul starts ~1.5us in); the Activation
queue carries h_t row-major prefetches and output stores.
"""

import numpy as np
import ml_dtypes

import concourse.bacc as bacc
import concourse.bass as bass
import concourse.mybir as mybir
import concourse.tile as tile
from concourse.bass_utils import run_bass_kernel_spmd

B, D = 32768, 896
N_CORES = 8
RPC = B // N_CORES            # rows per core = 4096
P = 128
KC = D // P                   # 7 K-chunks of 128
BLK = 512                     # rows per block
NBLK = RPC // BLK             # 8
BR = BLK // P                 # row-tiles per block = 4
NH = 448                      # stage-2 N chunk (2x448 = 896)
LN_EPS = 1e-5
RSQRT_MAGIC = 0x5F375A86      # fast inverse sqrt seed constant

F32 = mybir.dt.float32
BF16 = mybir.dt.bfloat16
FP8 = mybir.dt.float8e4
I32 = mybir.dt.int32

BF = ml_dtypes.bfloat16
E4 = ml_dtypes.float8_e4m3
# stage-1 weights are pre-scaled by S (exact power of 2) so the folded
# Bf term sits in fp8 e4m3's normal range; the silu eviction's activation
# scale multiplies by 1/S before the bias, undoing it exactly
S1 = 64.0

# test.py can flip these before calling kernel()
TRACE = False
_last_results = None


def _bcast_ap(ap, parts=P):
    return bass.AP(tensor=ap.tensor, offset=ap.offset, ap=[[0, parts], *ap.ap])


def _build(b2_zero: bool, ln_trivial: bool):
    nc = bacc.Bacc(None, target_bir_lowering=False)

    htT = nc.dram_tensor("htT", [D, RPC], BF16, kind="ExternalInput")
    pxT = nc.dram_tensor("pxT", [D, RPC], FP8, kind="ExternalInput")
    ht_row = nc.dram_tensor("ht_row", [RPC, D], BF16, kind="ExternalInput")
    AT_d = nc.dram_tensor("AT", [D, D], BF16, kind="ExternalInput")
    BfT_d = nc.dram_tensor("BfT", [D, D], FP8, kind="ExternalInput")
    w2T_d = nc.dram_tensor("w2T", [D, D], BF16, kind="ExternalInput")
    c_d = nc.dram_tensor("c_t", [P, KC], F32, kind="ExternalInput")
    if not b2_zero:
        b2_d = nc.dram_tensor("b2", [D], F32, kind="ExternalInput")
    if not ln_trivial:
        lng_d = nc.dram_tensor("ln_g", [D], F32, kind="ExternalInput")
        lnb_d = nc.dram_tensor("ln_b", [D], F32, kind="ExternalInput")
    out_d = nc.dram_tensor("out", [RPC, D], F32, kind="ExternalOutput")

    with tile.TileContext(nc) as tc:
        with (
            tc.tile_pool(name="wpool", bufs=1) as wpool,
            tc.tile_pool(name="xpool", bufs=6) as xpool,
            tc.tile_pool(name="gpool", bufs=2) as gpool,
            tc.tile_pool(name="spool", bufs=8) as spool,
            tc.tile_pool(name="hpool", bufs=4) as hpool,
            tc.tile_pool(name="opool", bufs=6) as opool,
            tc.tile_pool(name="ps1", bufs=2, space="PSUM") as ps1p,
            tc.tile_pool(name="ps2a", bufs=3, space="PSUM") as ps2ap,
            tc.tile_pool(name="ps2b", bufs=3, space="PSUM") as ps2bp,
        ):
            # ---- persistent weights ----
            wA = wpool.tile([P, KC, D], BF16)
            wB = wpool.tile([P, KC, D], FP8)
            w2 = wpool.tile([P, KC, D], BF16)
            AT_v = AT_d.rearrange("(kc p) n -> p kc n", p=P)
            BfT_v = BfT_d.rearrange("(kc p) n -> p kc n", p=P)
            w2T_v = w2T_d.rearrange("(kc p) n -> p kc n", p=P)
            cT = wpool.tile([P, KC], F32)
            magic_t = wpool.tile([P, 1], I32)
            nc.vector.memset(magic_t[:], RSQRT_MAGIC)
            if not b2_zero:
                b2b = wpool.tile([P, D], F32)
                nc.gpsimd.dma_start(out=b2b[:], in_=_bcast_ap(b2_d[:]))
            if not ln_trivial:
                lngb = wpool.tile([P, D], F32)
                nc.gpsimd.dma_start(out=lngb[:], in_=_bcast_ap(lng_d[:]))
                lnbb = wpool.tile([P, D], F32)
                nc.gpsimd.dma_start(out=lnbb[:], in_=_bcast_ap(lnb_d[:]))

            htT_v = htT.rearrange("(kc p) n -> p kc n", p=P)
            pxT_v = pxT.rearrange("(kc p) n -> p kc n", p=P)
            htR_v = ht_row.rearrange("(nb br p) d -> nb p br d", br=BR, p=P)

            def emit_loads(blk, xh, xp, htr):
                cs = slice(blk * BLK, (blk + 1) * BLK)
                if blk == 0:
                    # prologue split across both HWDGE queues (each has
                    # ~0.6us/DMA descriptor cost): weights on SP, activation
                    # chunks on the Act queue (no Act compute queued yet, so
                    # no sequencer head-of-line blocking). The first matmul
                    # needs only wA[:,0,0:128] + xh[:,0] — land those first.
                    nc.sync.dma_start(out=wA[:, 0, 0:P], in_=AT_v[:, 0, 0:P])
                    nc.scalar.dma_start(out=xh[:, 0], in_=htT_v[:, 0, cs])
                    nc.sync.dma_start(out=wA[:, 0, P:D], in_=AT_v[:, 0, P:D])
                    for k in range(1, KC):
                        nc.sync.dma_start(out=wA[:, k], in_=AT_v[:, k])
                        nc.scalar.dma_start(out=xh[:, k], in_=htT_v[:, k, cs])
                    for k in range(KC):
                        nc.sync.dma_start(out=wB[:, k], in_=BfT_v[:, k])
                        nc.scalar.dma_start(out=xp[:, k], in_=pxT_v[:, k, cs])
                    # cT is first needed by the silu burst at the end of
                    # block 0's stage 1 — keep it off the critical path;
                    # htr(b0) is issued later by the driver (needed ~30us in)
                    nc.scalar.dma_start(out=cT[:], in_=c_d[:])
                else:
                    # steady state: bulk loads on SP only (bulk DMAs on the
                    # Act queue block its sequencer ahead of the silus)
                    nc.sync.dma_start(out=xh[:], in_=htT_v[:, :, cs])
                    nc.sync.dma_start(out=xp[:], in_=pxT_v[:, :, cs])
                    nc.sync.dma_start(out=htr[:], in_=htR_v[blk])

            # ---- stage 1: y1T = A^T-chunks . htT + Bf^T-chunks . pxT ----
            def emit_stage1(blk, xh, xp, g1):
                if blk == 0:
                    # k-outer with all 7 m-chain PSUM banks open: PE does 7
                    # matmuls per arriving weight chunk, pacing the cold
                    # start at DMA speed instead of waiting for all chunks
                    # borrow one bank-sized slot per chain across the pools
                    pools = [ps1p, ps1p, ps2ap, ps2ap, ps2ap, ps2bp, ps2bp]
                    tags = ["ps1", "ps1", "ps2a", "ps2a", "ps2a", "ps2b", "ps2b"]
                    ps1s = [pools[m].tile([P, BLK], F32, tag=tags[m],
                                          name=f"ps1k_{m}") for m in range(KC)]
                    for k in range(KC):
                        for m in range(KC):
                            nc.tensor.matmul(ps1s[m][:], wA[:, k, m * P:(m + 1) * P],
                                             xh[:, k], start=(k == 0), stop=False)
                    for j in range(3):
                        ks = slice(2 * j, 2 * j + 2)
                        for m in range(KC):
                            nc.tensor.matmul(ps1s[m][:], wB[:, ks, m * P:(m + 1) * P],
                                             xp[:, ks], start=False, stop=False,
                                             perf_mode=mybir.MatmulPerfMode.DoubleRow)
                    for m in range(KC):
                        nc.tensor.matmul(ps1s[m][:], wB[:, KC - 1, m * P:(m + 1) * P],
                                         xp[:, KC - 1], start=False, stop=True)
                        nc.scalar.activation(
                            g1[:, m], ps1s[m][:],
                            mybir.ActivationFunctionType.Silu,
                            bias=cT[:, m:m + 1], scale=1.0 / S1)
                else:
                    for m in range(KC):
                        ms = slice(m * P, (m + 1) * P)
                        ps1 = ps1p.tile([P, BLK], F32, tag="ps1")
                        for k in range(KC):
                            nc.tensor.matmul(ps1[:], wA[:, k, ms], xh[:, k],
                                             start=(k == 0), stop=False)
                        for j in range(3):
                            ks = slice(2 * j, 2 * j + 2)
                            nc.tensor.matmul(ps1[:], wB[:, ks, ms], xp[:, ks],
                                             start=False, stop=False,
                                             perf_mode=mybir.MatmulPerfMode.DoubleRow)
                        nc.tensor.matmul(ps1[:], wB[:, KC - 1, ms], xp[:, KC - 1],
                                         start=False, stop=True)
                        # g1 = silu(y1/S1 + c), evicted by ScalarE, feature-major
                        nc.scalar.activation(g1[:, m], ps1[:],
                                             mybir.ActivationFunctionType.Silu,
                                             bias=cT[:, m:m + 1], scale=1.0 / S1)

            # ---- stage 2 + 3 per 128-row tile ----
            def emit_stage23(blk, g1, htr):
                for r in range(BR):
                    rows = slice(blk * BLK + r * P, blk * BLK + (r + 1) * P)
                    rs = slice(r * P, (r + 1) * P)
                    ps2a = ps2ap.tile([P, NH], F32, tag="ps2a")
                    ps2b = ps2bp.tile([P, NH], F32, tag="ps2b")
                    for k in range(KC):
                        nc.tensor.matmul(ps2a[:], g1[:, k, rs], w2[:, k, 0:NH],
                                         start=(k == 0), stop=(k == KC - 1))
                    for k in range(KC):
                        nc.tensor.matmul(ps2b[:], g1[:, k, rs], w2[:, k, NH:D],
                                         start=(k == 0), stop=(k == KC - 1))

                    if b2_zero:
                        y0, y1 = ps2a[:], ps2b[:]
                    else:
                        yb = opool.tile([P, D], F32, tag="yb")
                        nc.vector.tensor_add(yb[:, 0:NH], ps2a[:], b2b[:, 0:NH])
                        nc.vector.tensor_add(yb[:, NH:D], ps2b[:], b2b[:, NH:D])
                        y0, y1 = yb[:, 0:NH], yb[:, NH:D]

                    # LN stats on DVE
                    stats = spool.tile([P, 2, 6], F32, tag="stats")
                    nc.vector.bn_stats(out=stats[:, 0], in_=y0)
                    nc.vector.bn_stats(out=stats[:, 1], in_=y1)
                    mv = spool.tile([P, 2], F32, tag="mv")
                    nc.vector.bn_aggr(out=mv[:], in_=stats[:])

                    # rstd = 1/sqrt(var+eps): fast-inverse-sqrt seed + 2
                    # Newton iterations, on DVE (cheap small-op overhead)
                    t0 = spool.tile([P, 1], F32, tag="t0")
                    nc.vector.tensor_scalar(t0[:], mv[:, 1:2], LN_EPS, None,
                                            mybir.AluOpType.add)
                    t1 = spool.tile([P, 1], I32, tag="t1")
                    nc.vector.tensor_scalar(t1[:], t0.bitcast(I32)[:], 1, None,
                                            mybir.AluOpType.logical_shift_right)
                    yr = spool.tile([P, 1], F32, tag="yr")
                    nc.vector.tensor_sub(yr.bitcast(I32)[:], magic_t[:], t1[:])
                    for _ in range(2):
                        a = spool.tile([P, 1], F32, tag="nt")
                        nc.vector.tensor_mul(a[:], yr[:], yr[:])
                        nc.vector.tensor_mul(a[:], a[:], t0[:])
                        nc.vector.tensor_scalar(a[:], a[:], -0.5, 1.5,
                                                mybir.AluOpType.mult,
                                                mybir.AluOpType.add)
                        nc.vector.tensor_mul(yr[:], yr[:], a[:])
                    nmr = spool.tile([P, 1], F32, tag="nmr")
                    nc.vector.scalar_tensor_tensor(
                        out=nmr[:], in0=mv[:, 0:1], scalar=-1.0, in1=yr[:],
                        op0=mybir.AluOpType.mult, op1=mybir.AluOpType.mult)

                    # normalize: half0 on ScalarE (Identity: in*rstd + nmr),
                    # half1 on DVE (tensor_scalar) — parallel engine paths.
                    # The very last row-tile does both halves on DVE: the
                    # rsqrt chain lives there, so no cross-engine sem hop
                    # and no queueing behind rt2's Act work in the drain.
                    o = opool.tile([P, D], F32, tag="o")
                    nc.scalar.activation(o[:, 0:NH], y0,
                                         mybir.ActivationFunctionType.Identity,
                                         bias=nmr[:], scale=yr[:])
                    nc.vector.tensor_scalar(o[:, NH:D], y1, yr[:], nmr[:],
                                            mybir.AluOpType.mult,
                                            mybir.AluOpType.add)
                    if not ln_trivial:
                        nc.vector.tensor_mul(o[:], o[:], lngb[:])
                    # residual adds on Pool (keeps DVE free for the next
                    # row-tile's stats/rsqrt chain); the very last row-tile
                    # adds on DVE so the drain doesn't queue behind Pool
                    if blk == NBLK - 1 and r == BR - 1:
                        nc.vector.tensor_add(o[:, 0:NH], o[:, 0:NH], htr[:, r, 0:NH])
                        nc.vector.tensor_add(o[:, NH:D], o[:, NH:D], htr[:, r, NH:D])
                    else:
                        nc.gpsimd.tensor_add(o[:, 0:NH], o[:, 0:NH], htr[:, r, 0:NH])
                        nc.gpsimd.tensor_add(o[:, NH:D], o[:, NH:D], htr[:, r, NH:D])
                    if not ln_trivial:
                        nc.vector.tensor_add(o[:], o[:], lnbb[:])
                    if blk == NBLK - 1:
                        # last block: store halves as they complete
                        nc.sync.dma_start(out=out_d[rows, 0:NH], in_=o[:, 0:NH])
                        nc.sync.dma_start(out=out_d[rows, NH:D], in_=o[:, NH:D])
                    else:
                        nc.sync.dma_start(out=out_d[rows, :], in_=o[:])

            # block-level software pipeline: emit s1(b) before s2(b-1) so
            # the in-order PE stream always has independent matmul work
            # while the Act engine drains a block's silu evictions; loads
            # prefetch one block ahead, w2 after block-1's loads (first
            # needed at s2(b0), which runs after s1(b1))
            tiles = {}

            def alloc_and_load(blk):
                xh = xpool.tile([P, KC, BLK], BF16, name="xh")
                xp = xpool.tile([P, KC, BLK], FP8, name="xp")
                htr = hpool.tile([P, BR, D], BF16, name="htr")
                tiles[blk] = (xh, xp, htr)
                emit_loads(blk, xh, xp, htr)

            def run_stage1(blk):
                g1 = gpool.tile([P, KC, BLK], BF16, name="g1")
                tiles[blk] = (*tiles[blk], g1)
                emit_stage1(blk, tiles[blk][0], tiles[blk][1], g1)

            alloc_and_load(0)
            alloc_and_load(1)
            alloc_and_load(2)
            nc.sync.dma_start(out=tiles[0][2][:], in_=htR_v[0])
            for k in range(KC):
                nc.sync.dma_start(out=w2[:, k], in_=w2T_v[:, k])
            run_stage1(0)
            for blk in range(1, NBLK):
                if blk + 2 < NBLK:
                    alloc_and_load(blk + 2)
                run_stage1(blk)
                emit_stage23(blk - 1, tiles[blk - 1][3], tiles[blk - 1][2])
            emit_stage23(NBLK - 1, tiles[NBLK - 1][3], tiles[NBLK - 1][2])

    nc.compile()
    return nc


def prepare(h_t, phi_x, in_proj_w, in_proj_b, out_proj_w, out_proj_b,
            w1, b1, w2, b2, ln_g, ln_b):
    """Host-side folding + build; returns (nc, in_maps)."""
    # ---- host-side weight folding (fp64) ----
    Wv = in_proj_w[2 * D:].astype(np.float64)
    bv = in_proj_b[2 * D:].astype(np.float64)
    Wo = out_proj_w.astype(np.float64)
    W1a = w1[:, :D].astype(np.float64)
    W1b = w1[:, D:].astype(np.float64)
    WoWv = Wo @ Wv
    Bf = W1b @ WoWv
    c = b1.astype(np.float64) + W1b @ (Wo @ bv + out_proj_b.astype(np.float64))

    AT = np.ascontiguousarray(W1a.T * S1).astype(BF)
    BfT = np.ascontiguousarray(Bf.T * S1).astype(E4)
    w2T = np.ascontiguousarray(np.asarray(w2, np.float64).T).astype(BF)
    c_t = np.ascontiguousarray(c.reshape(KC, P).T).astype(np.float32)

    b2_zero = bool(np.all(b2 == 0))
    ln_trivial = bool(np.all(ln_g == 1) and np.all(ln_b == 0))

    nc = _build(b2_zero, ln_trivial)

    h_t = np.asarray(h_t, dtype=np.float32)
    phi_x = np.asarray(phi_x, dtype=np.float32)

    in_maps = []
    for i in range(N_CORES):
        rows = slice(i * RPC, (i + 1) * RPC)
        ht_i = h_t[rows]
        px_i = phi_x[rows]
        m = {
            "htT": np.ascontiguousarray(ht_i.T).astype(BF),
            "pxT": np.ascontiguousarray(px_i.T).astype(E4),
            "ht_row": ht_i.astype(BF),
            "AT": AT,
            "BfT": BfT,
            "w2T": w2T,
            "c_t": c_t,
        }
        if not b2_zero:
            m["b2"] = np.asarray(b2, dtype=np.float32)
        if not ln_trivial:
            m["ln_g"] = np.asarray(ln_g, dtype=np.float32)
            m["ln_b"] = np.asarray(ln_b, dtype=np.float32)
        in_maps.append(m)

    return nc, in_maps


def kernel(**inputs):
    global _last_results
    nc, in_maps = prepare(**inputs)
    res = run_bass_kernel_spmd(nc, in_maps, core_ids=list(range(N_CORES)),
                               trace=TRACE)
    _last_results = res
    return np.concatenate([r["out"] for r in res.results], axis=0)


# revision 77
# speedup vs baseline: 1.2439x; 1.0266x over previous
"""Fused LyapunovThinkingBlock kernel for 8x TRN2 NeuronCores.

Math (B=32768, D=896): the reference block is
    q,k unused: softmax over a length-1 axis is exactly 1.0 => ctx == v
    v     = phi_x @ Wv^T + b_v
    h_att = v @ Wo^T + b_o
    g1    = silu([h_t, h_att] @ w1^T + b1)
    g2    = g1 @ w2^T + b2
    out   = h_t + LN(g2) * ln_g + ln_b

Weight folding (host, fp64):
    h_att = phi_x @ (Wo Wv)^T + (Wo b_v + b_o)
    [h_t, h_att] @ w1^T = h_t @ W1a^T + h_att @ W1b^T   (w1 = [W1a | W1b])
    => g1 = silu(h_t @ W1a^T + phi_x @ Bf^T + c)
       Bf = W1b Wo Wv,  c = b1 + W1b (Wo b_v + b_o)

Device (pure data parallel, batch sharded over 8 cores, 4096 rows each):
    stage 1 (feature-major): y1T[d,r] accumulated over 14 K-chunk matmuls,
        silu+bias on ScalarE directly from PSUM -> g1T (bf16) in SBUF
    stage 2 (row-major): y2[r,d] = g1 @ w2^T via activation-as-stationary
    stage 3: LayerNorm (bn_stats on DVE, fast-rsqrt Newton chain on Pool,
        normalize on ScalarE, residual add on Pool), store.

All matmul operands are bf16 (1 cycle/row, same as fp32r, half the DMA).
DMA queues: SP carries the critical weight+activation load stream with
per-K-chunk granularity (first matmul starts ~1.5us in); the Activation
queue carries h_t row-major prefetches and output stores.
"""

import numpy as np
import ml_dtypes

import concourse.bacc as bacc
import concourse.bass as bass
import concourse.mybir as mybir
import concourse.tile as tile
from concourse.bass_utils import run_bass_kernel_spmd

B, D = 32768, 896
N_CORES = 8
RPC = B // N_CORES            # rows per core = 4096
P = 128
KC = D // P                   # 7 K-chunks of 128
BLK = 512                     # rows per block
NBLK = RPC // BLK             # 8
BR = BLK // P                 # row-tiles per block = 4
NH = 448                      # stage-2 N chunk (2x448 = 896)
LN_EPS = 1e-5
RSQRT_MAGIC = 0x5F375A86      # fast inverse sqrt seed constant

F32 = mybir.dt.float32
BF16 = mybir.dt.bfloat16
FP8 = mybir.dt.float8e4
I32 = mybir.dt.int32

BF = ml_dtypes.bfloat16
E4 = ml_dtypes.float8_e4m3
# stage-1 weights are pre-scaled by S (exact power of 2) so the folded
# Bf term sits in fp8 e4m3's normal range; the silu eviction's activation
# scale multiplies by 1/S before the bias, undoing it exactly
S1 = 64.0

# test.py can flip these before calling kernel()
TRACE = False
_last_results = None


def _bcast_ap(ap, parts=P):
    return bass.AP(tensor=ap.tensor, offset=ap.offset, ap=[[0, parts], *ap.ap])


def _build(b2_zero: bool, ln_trivial: bool):
    nc = bacc.Bacc(None, target_bir_lowering=False)

    htT = nc.dram_tensor("htT", [D, RPC], BF16, kind="ExternalInput")
    pxT = nc.dram_tensor("pxT", [D, RPC], FP8, kind="ExternalInput")
    ht_row = nc.dram_tensor("ht_row", [RPC, D], BF16, kind="ExternalInput")
    AT_d = nc.dram_tensor("AT", [D, D], BF16, kind="ExternalInput")
    BfT_d = nc.dram_tensor("BfT", [D, D], FP8, kind="ExternalInput")
    w2T_d = nc.dram_tensor("w2T", [D, D], BF16, kind="ExternalInput")
    c_d = nc.dram_tensor("c_t", [P, KC], F32, kind="ExternalInput")
    if not b2_zero:
        b2_d = nc.dram_tensor("b2", [D], F32, kind="ExternalInput")
    if not ln_trivial:
        lng_d = nc.dram_tensor("ln_g", [D], F32, kind="ExternalInput")
        lnb_d = nc.dram_tensor("ln_b", [D], F32, kind="ExternalInput")
    out_d = nc.dram_tensor("out", [RPC, D], F32, kind="ExternalOutput")

    with tile.TileContext(nc) as tc:
        with (
            tc.tile_pool(name="wpool", bufs=1) as wpool,
            tc.tile_pool(name="xpool", bufs=6) as xpool,
            tc.tile_pool(name="gpool", bufs=2) as gpool,
            tc.tile_pool(name="spool", bufs=8) as spool,
            tc.tile_pool(name="hpool", bufs=4) as hpool,
            tc.tile_pool(name="opool", bufs=6) as opool,
            tc.tile_pool(name="ps1", bufs=2, space="PSUM") as ps1p,
            tc.tile_pool(name="ps2a", bufs=3, space="PSUM") as ps2ap,
            tc.tile_pool(name="ps2b", bufs=3, space="PSUM") as ps2bp,
        ):
            # ---- persistent weights ----
            wA = wpool.tile([P, KC, D], BF16)
            wB = wpool.tile([P, KC, D], FP8)
            w2 = wpool.tile([P, KC, D], BF16)
            AT_v = AT_d.rearrange("(kc p) n -> p kc n", p=P)
            BfT_v = BfT_d.rearrange("(kc p) n -> p kc n", p=P)
            w2T_v = w2T_d.rearrange("(kc p) n -> p kc n", p=P)
            cT = wpool.tile([P, KC], F32)
            magic_t = wpool.tile([P, 1], I32)
            nc.vector.memset(magic_t[:], RSQRT_MAGIC)
            if not b2_zero:
                b2b = wpool.tile([P, D], F32)
                nc.gpsimd.dma_start(out=b2b[:], in_=_bcast_ap(b2_d[:]))
            if not ln_trivial:
                lngb = wpool.tile([P, D], F32)
                nc.gpsimd.dma_start(out=lngb[:], in_=_bcast_ap(lng_d[:]))
                lnbb = wpool.tile([P, D], F32)
                nc.gpsimd.dma_start(out=lnbb[:], in_=_bcast_ap(lnb_d[:]))

            htT_v = htT.rearrange("(kc p) n -> p kc n", p=P)
            pxT_v = pxT.rearrange("(kc p) n -> p kc n", p=P)
            htR_v = ht_row.rearrange("(nb br p) d -> nb p br d", br=BR, p=P)

            def emit_loads(blk, xh, xp, htr):
                cs = slice(blk * BLK, (blk + 1) * BLK)
                if blk == 0:
                    # prologue split across both HWDGE queues (each has
                    # ~0.6us/DMA descriptor cost): weights on SP, activation
                    # chunks on the Act queue (no Act compute queued yet, so
                    # no sequencer head-of-line blocking). The first matmul
                    # needs only wA[:,0,0:128] + xh[:,0] — land those first.
                    nc.sync.dma_start(out=wA[:, 0, 0:P], in_=AT_v[:, 0, 0:P])
                    nc.scalar.dma_start(out=xh[:, 0], in_=htT_v[:, 0, cs])
                    nc.sync.dma_start(out=wA[:, 0, P:D], in_=AT_v[:, 0, P:D])
                    for k in range(1, KC):
                        nc.sync.dma_start(out=wA[:, k], in_=AT_v[:, k])
                        nc.scalar.dma_start(out=xh[:, k], in_=htT_v[:, k, cs])
                    # wB/xp are fp8 — small enough that per-chunk DMAs are
                    # HWDGE-descriptor-bound; load each as ONE bulk DMA
                    # (they land during the compute-bound wA phase)
                    nc.sync.dma_start(out=wB[:], in_=BfT_v[:])
                    nc.scalar.dma_start(out=xp[:], in_=pxT_v[:, :, cs])
                    # cT is first needed by the silu burst at the end of
                    # block 0's stage 1 — keep it off the critical path;
                    # htr(b0) is issued later by the driver (needed ~30us in)
                    nc.scalar.dma_start(out=cT[:], in_=c_d[:])
                else:
                    # steady state: bulk loads on SP only (bulk DMAs on the
                    # Act queue block its sequencer ahead of the silus)
                    nc.sync.dma_start(out=xh[:], in_=htT_v[:, :, cs])
                    nc.sync.dma_start(out=xp[:], in_=pxT_v[:, :, cs])
                    nc.sync.dma_start(out=htr[:], in_=htR_v[blk])

            # ---- stage 1: y1T = A^T-chunks . htT + Bf^T-chunks . pxT ----
            def emit_stage1(blk, xh, xp, g1):
                if blk == 0:
                    # k-outer with all 7 m-chain PSUM banks open: PE does 7
                    # matmuls per arriving weight chunk, pacing the cold
                    # start at DMA speed instead of waiting for all chunks
                    # borrow one bank-sized slot per chain across the pools
                    # leave one ps1 slot unborrowed: block 1's first m-chain
                    # can then start without waiting on block-0's silu burst
                    pools = [ps1p, ps2ap, ps2ap, ps2ap, ps2bp, ps2bp, ps2bp]
                    tags = ["ps1", "ps2a", "ps2a", "ps2a", "ps2b", "ps2b", "ps2b"]
                    ps1s = [pools[m].tile([P, BLK], F32, tag=tags[m],
                                          name=f"ps1k_{m}") for m in range(KC)]
                    for k in range(KC):
                        for m in range(KC):
                            nc.tensor.matmul(ps1s[m][:], wA[:, k, m * P:(m + 1) * P],
                                             xh[:, k], start=(k == 0), stop=False)
                    for j in range(3):
                        ks = slice(2 * j, 2 * j + 2)
                        for m in range(KC):
                            nc.tensor.matmul(ps1s[m][:], wB[:, ks, m * P:(m + 1) * P],
                                             xp[:, ks], start=False, stop=False,
                                             perf_mode=mybir.MatmulPerfMode.DoubleRow)
                    for m in range(KC):
                        nc.tensor.matmul(ps1s[m][:], wB[:, KC - 1, m * P:(m + 1) * P],
                                         xp[:, KC - 1], start=False, stop=True)
                        nc.scalar.activation(
                            g1[:, m], ps1s[m][:],
                            mybir.ActivationFunctionType.Silu,
                            bias=cT[:, m:m + 1], scale=1.0 / S1)
                else:
                    for m in range(KC):
                        ms = slice(m * P, (m + 1) * P)
                        ps1 = ps1p.tile([P, BLK], F32, tag="ps1")
                        for k in range(KC):
                            nc.tensor.matmul(ps1[:], wA[:, k, ms], xh[:, k],
                                             start=(k == 0), stop=False)
                        for j in range(3):
                            ks = slice(2 * j, 2 * j + 2)
                            nc.tensor.matmul(ps1[:], wB[:, ks, ms], xp[:, ks],
                                             start=False, stop=False,
                                             perf_mode=mybir.MatmulPerfMode.DoubleRow)
                        nc.tensor.matmul(ps1[:], wB[:, KC - 1, ms], xp[:, KC - 1],
                                         start=False, stop=True)
                        # g1 = silu(y1/S1 + c), evicted by ScalarE, feature-major
                        nc.scalar.activation(g1[:, m], ps1[:],
                                             mybir.ActivationFunctionType.Silu,
                                             bias=cT[:, m:m + 1], scale=1.0 / S1)

            # ---- stage 2 + 3 per 128-row tile ----
            def emit_stage23(blk, g1, htr):
                for r in range(BR):
                    rows = slice(blk * BLK + r * P, blk * BLK + (r + 1) * P)
                    rs = slice(r * P, (r + 1) * P)
                    ps2a = ps2ap.tile([P, NH], F32, tag="ps2a")
                    ps2b = ps2bp.tile([P, NH], F32, tag="ps2b")
                    for k in range(KC):
                        nc.tensor.matmul(ps2a[:], g1[:, k, rs], w2[:, k, 0:NH],
                                         start=(k == 0), stop=(k == KC - 1))
                    for k in range(KC):
                        nc.tensor.matmul(ps2b[:], g1[:, k, rs], w2[:, k, NH:D],
                                         start=(k == 0), stop=(k == KC - 1))

                    if b2_zero:
                        y0, y1 = ps2a[:], ps2b[:]
                    else:
                        yb = opool.tile([P, D], F32, tag="yb")
                        nc.vector.tensor_add(yb[:, 0:NH], ps2a[:], b2b[:, 0:NH])
                        nc.vector.tensor_add(yb[:, NH:D], ps2b[:], b2b[:, NH:D])
                        y0, y1 = yb[:, 0:NH], yb[:, NH:D]

                    # LN stats on DVE
                    stats = spool.tile([P, 2, 6], F32, tag="stats")
                    nc.vector.bn_stats(out=stats[:, 0], in_=y0)
                    nc.vector.bn_stats(out=stats[:, 1], in_=y1)
                    mv = spool.tile([P, 2], F32, tag="mv")
                    nc.vector.bn_aggr(out=mv[:], in_=stats[:])

                    # rstd = 1/sqrt(var+eps): fast-inverse-sqrt seed + 2
                    # Newton iterations, on DVE (cheap small-op overhead)
                    t0 = spool.tile([P, 1], F32, tag="t0")
                    nc.vector.tensor_scalar(t0[:], mv[:, 1:2], LN_EPS, None,
                                            mybir.AluOpType.add)
                    t1 = spool.tile([P, 1], I32, tag="t1")
                    nc.vector.tensor_scalar(t1[:], t0.bitcast(I32)[:], 1, None,
                                            mybir.AluOpType.logical_shift_right)
                    yr = spool.tile([P, 1], F32, tag="yr")
                    nc.vector.tensor_sub(yr.bitcast(I32)[:], magic_t[:], t1[:])
                    for _ in range(2):
                        a = spool.tile([P, 1], F32, tag="nt")
                        nc.vector.tensor_mul(a[:], yr[:], yr[:])
                        nc.vector.tensor_mul(a[:], a[:], t0[:])
                        nc.vector.tensor_scalar(a[:], a[:], -0.5, 1.5,
                                                mybir.AluOpType.mult,
                                                mybir.AluOpType.add)
                        nc.vector.tensor_mul(yr[:], yr[:], a[:])
                    nmr = spool.tile([P, 1], F32, tag="nmr")
                    nc.vector.scalar_tensor_tensor(
                        out=nmr[:], in0=mv[:, 0:1], scalar=-1.0, in1=yr[:],
                        op0=mybir.AluOpType.mult, op1=mybir.AluOpType.mult)

                    # normalize: half0 on ScalarE (Identity: in*rstd + nmr),
                    # half1 on DVE (tensor_scalar) — parallel engine paths.
                    # The very last row-tile does both halves on DVE: the
                    # rsqrt chain lives there, so no cross-engine sem hop
                    # and no queueing behind rt2's Act work in the drain.
                    o = opool.tile([P, D], F32, tag="o")
                    nc.scalar.activation(o[:, 0:NH], y0,
                                         mybir.ActivationFunctionType.Identity,
                                         bias=nmr[:], scale=yr[:])
                    nc.vector.tensor_scalar(o[:, NH:D], y1, yr[:], nmr[:],
                                            mybir.AluOpType.mult,
                                            mybir.AluOpType.add)
                    if not ln_trivial:
                        nc.vector.tensor_mul(o[:], o[:], lngb[:])
                    # residual adds on Pool (keeps DVE free for the next
                    # row-tile's stats/rsqrt chain); the very last row-tile
                    # adds on DVE so the drain doesn't queue behind Pool
                    if blk == NBLK - 1 and r == BR - 1:
                        nc.vector.tensor_add(o[:, 0:NH], o[:, 0:NH], htr[:, r, 0:NH])
                        nc.vector.tensor_add(o[:, NH:D], o[:, NH:D], htr[:, r, NH:D])
                    else:
                        nc.gpsimd.tensor_add(o[:, 0:NH], o[:, 0:NH], htr[:, r, 0:NH])
                        nc.gpsimd.tensor_add(o[:, NH:D], o[:, NH:D], htr[:, r, NH:D])
                    if not ln_trivial:
                        nc.vector.tensor_add(o[:], o[:], lnbb[:])
                    if blk == NBLK - 1:
                        # last block: store halves as they complete
                        nc.sync.dma_start(out=out_d[rows, 0:NH], in_=o[:, 0:NH])
                        nc.sync.dma_start(out=out_d[rows, NH:D], in_=o[:, NH:D])
                    else:
                        nc.sync.dma_start(out=out_d[rows, :], in_=o[:])

            # block-level software pipeline: emit s1(b) before s2(b-1) so
            # the in-order PE stream always has independent matmul work
            # while the Act engine drains a block's silu evictions; loads
            # prefetch one block ahead, w2 after block-1's loads (first
            # needed at s2(b0), which runs after s1(b1))
            tiles = {}

            def alloc_and_load(blk):
                xh = xpool.tile([P, KC, BLK], BF16, name="xh")
                xp = xpool.tile([P, KC, BLK], FP8, name="xp")
                htr = hpool.tile([P, BR, D], BF16, name="htr")
                tiles[blk] = (xh, xp, htr)
                emit_loads(blk, xh, xp, htr)

            def run_stage1(blk):
                g1 = gpool.tile([P, KC, BLK], BF16, name="g1")
                tiles[blk] = (*tiles[blk], g1)
                emit_stage1(blk, tiles[blk][0], tiles[blk][1], g1)

            alloc_and_load(0)
            alloc_and_load(1)
            alloc_and_load(2)
            nc.sync.dma_start(out=tiles[0][2][:], in_=htR_v[0])
            for k in range(KC):
                nc.sync.dma_start(out=w2[:, k], in_=w2T_v[:, k])
            run_stage1(0)
            for blk in range(1, NBLK):
                if blk + 2 < NBLK:
                    alloc_and_load(blk + 2)
                run_stage1(blk)
                emit_stage23(blk - 1, tiles[blk - 1][3], tiles[blk - 1][2])
            emit_stage23(NBLK - 1, tiles[NBLK - 1][3], tiles[NBLK - 1][2])

    nc.compile()
    return nc


def prepare(h_t, phi_x, in_proj_w, in_proj_b, out_proj_w, out_proj_b,
            w1, b1, w2, b2, ln_g, ln_b):
    """Host-side folding + build; returns (nc, in_maps)."""
    # ---- host-side weight folding (fp64) ----
    Wv = in_proj_w[2 * D:].astype(np.float64)
    bv = in_proj_b[2 * D:].astype(np.float64)
    Wo = out_proj_w.astype(np.float64)
    W1a = w1[:, :D].astype(np.float64)
    W1b = w1[:, D:].astype(np.float64)
    WoWv = Wo @ Wv
    Bf = W1b @ WoWv
    c = b1.astype(np.float64) + W1b @ (Wo @ bv + out_proj_b.astype(np.float64))

    AT = np.ascontiguousarray(W1a.T * S1).astype(BF)
    BfT = np.ascontiguousarray(Bf.T * S1).astype(E4)
    w2T = np.ascontiguousarray(np.asarray(w2, np.float64).T).astype(BF)
    c_t = np.ascontiguousarray(c.reshape(KC, P).T).astype(np.float32)

    b2_zero = bool(np.all(b2 == 0))
    ln_trivial = bool(np.all(ln_g == 1) and np.all(ln_b == 0))

    nc = _build(b2_zero, ln_trivial)

    h_t = np.asarray(h_t, dtype=np.float32)
    phi_x = np.asarray(phi_x, dtype=np.float32)

    in_maps = []
    for i in range(N_CORES):
        rows = slice(i * RPC, (i + 1) * RPC)
        ht_i = h_t[rows]
        px_i = phi_x[rows]
        m = {
            "htT": np.ascontiguousarray(ht_i.T).astype(BF),
            "pxT": np.ascontiguousarray(px_i.T).astype(E4),
            "ht_row": ht_i.astype(BF),
            "AT": AT,
            "BfT": BfT,
            "w2T": w2T,
            "c_t": c_t,
        }
        if not b2_zero:
            m["b2"] = np.asarray(b2, dtype=np.float32)
        if not ln_trivial:
            m["ln_g"] = np.asarray(ln_g, dtype=np.float32)
            m["ln_b"] = np.asarray(ln_b, dtype=np.float32)
        in_maps.append(m)

    return nc, in_maps


def kernel(**inputs):
    global _last_results
    nc, in_maps = prepare(**inputs)
    res = run_bass_kernel_spmd(nc, in_maps, core_ids=list(range(N_CORES)),
                               trace=TRACE)
    _last_results = res
    return np.concatenate([r["out"] for r in res.results], axis=0)


# revision 83
# speedup vs baseline: 1.2484x; 1.0037x over previous
"""Fused LyapunovThinkingBlock kernel for 8x TRN2 NeuronCores.

Math (B=32768, D=896): the reference block is
    q,k unused: softmax over a length-1 axis is exactly 1.0 => ctx == v
    v     = phi_x @ Wv^T + b_v
    h_att = v @ Wo^T + b_o
    g1    = silu([h_t, h_att] @ w1^T + b1)
    g2    = g1 @ w2^T + b2
    out   = h_t + LN(g2) * ln_g + ln_b

Weight folding (host, fp64):
    h_att = phi_x @ (Wo Wv)^T + (Wo b_v + b_o)
    [h_t, h_att] @ w1^T = h_t @ W1a^T + h_att @ W1b^T   (w1 = [W1a | W1b])
    => g1 = silu(h_t @ W1a^T + phi_x @ Bf^T + c)
       Bf = W1b Wo Wv,  c = b1 + W1b (Wo b_v + b_o)

Device (pure data parallel, batch sharded over 8 cores, 4096 rows each):
    stage 1 (feature-major): y1T[d,r] accumulated over 14 K-chunk matmuls,
        silu+bias on ScalarE directly from PSUM -> g1T (bf16) in SBUF
    stage 2 (row-major): y2[r,d] = g1 @ w2^T via activation-as-stationary
    stage 3: LayerNorm (bn_stats on DVE, fast-rsqrt Newton chain on Pool,
        normalize on ScalarE, residual add on Pool), store.

All matmul operands are bf16 (1 cycle/row, same as fp32r, half the DMA).
DMA queues: SP carries the critical weight+activation load stream with
per-K-chunk granularity (first matmul starts ~1.5us in); the Activation
queue carries h_t row-major prefetches and output stores.
"""

import numpy as np
import ml_dtypes

import concourse.bacc as bacc
import concourse.bass as bass
import concourse.mybir as mybir
import concourse.tile as tile
from concourse.bass_utils import run_bass_kernel_spmd

B, D = 32768, 896
N_CORES = 8
RPC = B // N_CORES            # rows per core = 4096
P = 128
KC = D // P                   # 7 K-chunks of 128
BLK = 512                     # rows per block
NBLK = RPC // BLK             # 8
BR = BLK // P                 # row-tiles per block = 4
NH = 448                      # stage-2 N chunk (2x448 = 896)
LN_EPS = 1e-5
RSQRT_MAGIC = 0x5F375A86      # fast inverse sqrt seed constant

F32 = mybir.dt.float32
BF16 = mybir.dt.bfloat16
FP8 = mybir.dt.float8e4
I32 = mybir.dt.int32

BF = ml_dtypes.bfloat16
E4 = ml_dtypes.float8_e4m3
# stage-1 weights are pre-scaled by S (exact power of 2) so the folded
# Bf term sits in fp8 e4m3's normal range; the silu eviction's activation
# scale multiplies by 1/S before the bias, undoing it exactly
S1 = 64.0

# test.py can flip these before calling kernel()
TRACE = False
_last_results = None


def _bcast_ap(ap, parts=P):
    return bass.AP(tensor=ap.tensor, offset=ap.offset, ap=[[0, parts], *ap.ap])


def _build(b2_zero: bool, ln_trivial: bool):
    nc = bacc.Bacc(None, target_bir_lowering=False)

    htT = nc.dram_tensor("htT", [D, RPC], BF16, kind="ExternalInput")
    pxT = nc.dram_tensor("pxT", [D, RPC], FP8, kind="ExternalInput")
    ht_row = nc.dram_tensor("ht_row", [RPC, D], BF16, kind="ExternalInput")
    AT_d = nc.dram_tensor("AT", [D, D], BF16, kind="ExternalInput")
    BfT_d = nc.dram_tensor("BfT", [D, D], FP8, kind="ExternalInput")
    w2T_d = nc.dram_tensor("w2T", [D, D], BF16, kind="ExternalInput")
    c_d = nc.dram_tensor("c_t", [P, KC], F32, kind="ExternalInput")
    if not b2_zero:
        b2_d = nc.dram_tensor("b2", [D], F32, kind="ExternalInput")
    if not ln_trivial:
        lng_d = nc.dram_tensor("ln_g", [D], F32, kind="ExternalInput")
        lnb_d = nc.dram_tensor("ln_b", [D], F32, kind="ExternalInput")
    out_d = nc.dram_tensor("out", [RPC, D], F32, kind="ExternalOutput")

    with tile.TileContext(nc) as tc:
        with (
            tc.tile_pool(name="wpool", bufs=1) as wpool,
            tc.tile_pool(name="xpool", bufs=6) as xpool,
            tc.tile_pool(name="gpool", bufs=2) as gpool,
            tc.tile_pool(name="spool", bufs=8) as spool,
            tc.tile_pool(name="hpool", bufs=4) as hpool,
            tc.tile_pool(name="opool", bufs=6) as opool,
            tc.tile_pool(name="ps1", bufs=2, space="PSUM") as ps1p,
            tc.tile_pool(name="ps2a", bufs=3, space="PSUM") as ps2ap,
            tc.tile_pool(name="ps2b", bufs=3, space="PSUM") as ps2bp,
        ):
            # ---- persistent weights ----
            wA = wpool.tile([P, KC, D], BF16)
            wB = wpool.tile([P, KC, D], FP8)
            w2 = wpool.tile([P, KC, D], BF16)
            AT_v = AT_d.rearrange("(kc p) n -> p kc n", p=P)
            BfT_v = BfT_d.rearrange("(kc p) n -> p kc n", p=P)
            w2T_v = w2T_d.rearrange("(kc p) n -> p kc n", p=P)
            cT = wpool.tile([P, KC], F32)
            magic_t = wpool.tile([P, 1], I32)
            nc.vector.memset(magic_t[:], RSQRT_MAGIC)
            if not b2_zero:
                b2b = wpool.tile([P, D], F32)
                nc.gpsimd.dma_start(out=b2b[:], in_=_bcast_ap(b2_d[:]))
            if not ln_trivial:
                lngb = wpool.tile([P, D], F32)
                nc.gpsimd.dma_start(out=lngb[:], in_=_bcast_ap(lng_d[:]))
                lnbb = wpool.tile([P, D], F32)
                nc.gpsimd.dma_start(out=lnbb[:], in_=_bcast_ap(lnb_d[:]))

            htT_v = htT.rearrange("(kc p) n -> p kc n", p=P)
            pxT_v = pxT.rearrange("(kc p) n -> p kc n", p=P)
            htR_v = ht_row.rearrange("(nb br p) d -> nb p br d", br=BR, p=P)

            def emit_loads(blk, xh, xp, htr):
                cs = slice(blk * BLK, (blk + 1) * BLK)
                if blk == 0:
                    # prologue split across both HWDGE queues (each has
                    # ~0.6us/DMA descriptor cost): weights on SP, activation
                    # chunks on the Act queue (no Act compute queued yet, so
                    # no sequencer head-of-line blocking). The first matmul
                    # needs only wA[:,0,0:128] + xh[:,0] — land those first.
                    nc.sync.dma_start(out=wA[:, 0, 0:P], in_=AT_v[:, 0, 0:P])
                    nc.scalar.dma_start(out=xh[:, 0], in_=htT_v[:, 0, cs])
                    nc.sync.dma_start(out=wA[:, 0, P:D], in_=AT_v[:, 0, P:D])
                    for k in range(1, KC):
                        nc.sync.dma_start(out=wA[:, k], in_=AT_v[:, k])
                        nc.scalar.dma_start(out=xh[:, k], in_=htT_v[:, k, cs])
                    # wB/xp are fp8 — small enough that per-chunk DMAs are
                    # HWDGE-descriptor-bound; load each as ONE bulk DMA
                    # (they land during the compute-bound wA phase)
                    nc.sync.dma_start(out=wB[:], in_=BfT_v[:])
                    nc.scalar.dma_start(out=xp[:], in_=pxT_v[:, :, cs])
                    # cT is first needed by the silu burst at the end of
                    # block 0's stage 1 — keep it off the critical path;
                    # htr(b0) is issued later by the driver (needed ~30us in)
                    nc.scalar.dma_start(out=cT[:], in_=c_d[:])
                else:
                    # steady state: bulk loads on SP only (bulk DMAs on the
                    # Act queue block its sequencer ahead of the silus)
                    nc.sync.dma_start(out=xh[:], in_=htT_v[:, :, cs])
                    nc.sync.dma_start(out=xp[:], in_=pxT_v[:, :, cs])
                    nc.sync.dma_start(out=htr[:], in_=htR_v[blk])

            # ---- stage 1: y1T = A^T-chunks . htT + Bf^T-chunks . pxT ----
            def emit_stage1(blk, xh, xp, g1):
                if blk == 0:
                    # k-outer with all 7 m-chain PSUM banks open: PE does 7
                    # matmuls per arriving weight chunk, pacing the cold
                    # start at DMA speed instead of waiting for all chunks
                    # borrow one bank-sized slot per chain across the pools
                    # leave one ps1 slot unborrowed: block 1's first m-chain
                    # can then start without waiting on block-0's silu burst
                    pools = [ps1p, ps2ap, ps2ap, ps2ap, ps2bp, ps2bp, ps2bp]
                    tags = ["ps1", "ps2a", "ps2a", "ps2a", "ps2b", "ps2b", "ps2b"]
                    ps1s = [pools[m].tile([P, BLK], F32, tag=tags[m],
                                          name=f"ps1k_{m}") for m in range(KC)]
                    for k in range(KC):
                        for m in range(KC):
                            nc.tensor.matmul(ps1s[m][:], wA[:, k, m * P:(m + 1) * P],
                                             xh[:, k], start=(k == 0), stop=False)
                    for j in range(3):
                        ks = slice(2 * j, 2 * j + 2)
                        for m in range(KC):
                            nc.tensor.matmul(ps1s[m][:], wB[:, ks, m * P:(m + 1) * P],
                                             xp[:, ks], start=False, stop=False,
                                             perf_mode=mybir.MatmulPerfMode.DoubleRow)
                    for m in range(KC):
                        nc.tensor.matmul(ps1s[m][:], wB[:, KC - 1, m * P:(m + 1) * P],
                                         xp[:, KC - 1], start=False, stop=True)
                        nc.scalar.activation(
                            g1[:, m], ps1s[m][:],
                            mybir.ActivationFunctionType.Silu,
                            bias=cT[:, m:m + 1], scale=1.0 / S1)
                else:
                    for m in range(KC):
                        ms = slice(m * P, (m + 1) * P)
                        ps1 = ps1p.tile([P, BLK], F32, tag="ps1")
                        for k in range(KC):
                            nc.tensor.matmul(ps1[:], wA[:, k, ms], xh[:, k],
                                             start=(k == 0), stop=False)
                        for j in range(3):
                            ks = slice(2 * j, 2 * j + 2)
                            nc.tensor.matmul(ps1[:], wB[:, ks, ms], xp[:, ks],
                                             start=False, stop=False,
                                             perf_mode=mybir.MatmulPerfMode.DoubleRow)
                        nc.tensor.matmul(ps1[:], wB[:, KC - 1, ms], xp[:, KC - 1],
                                         start=False, stop=True)
                        # g1 = silu(y1/S1 + c), evicted by ScalarE, feature-major
                        nc.scalar.activation(g1[:, m], ps1[:],
                                             mybir.ActivationFunctionType.Silu,
                                             bias=cT[:, m:m + 1], scale=1.0 / S1)

            # ---- stage 2 + 3 per 128-row tile ----
            def emit_stage23(blk, g1, htr):
                for r in range(BR):
                    rows = slice(blk * BLK + r * P, blk * BLK + (r + 1) * P)
                    rs = slice(r * P, (r + 1) * P)
                    ps2a = ps2ap.tile([P, NH], F32, tag="ps2a")
                    ps2b = ps2bp.tile([P, NH], F32, tag="ps2b")
                    for k in range(KC):
                        nc.tensor.matmul(ps2a[:], g1[:, k, rs], w2[:, k, 0:NH],
                                         start=(k == 0), stop=(k == KC - 1))
                    for k in range(KC):
                        nc.tensor.matmul(ps2b[:], g1[:, k, rs], w2[:, k, NH:D],
                                         start=(k == 0), stop=(k == KC - 1))

                    if b2_zero:
                        y0, y1 = ps2a[:], ps2b[:]
                    else:
                        yb = opool.tile([P, D], F32, tag="yb")
                        nc.vector.tensor_add(yb[:, 0:NH], ps2a[:], b2b[:, 0:NH])
                        nc.vector.tensor_add(yb[:, NH:D], ps2b[:], b2b[:, NH:D])
                        y0, y1 = yb[:, 0:NH], yb[:, NH:D]

                    # LN stats on DVE
                    stats = spool.tile([P, 2, 6], F32, tag="stats")
                    nc.vector.bn_stats(out=stats[:, 0], in_=y0)
                    nc.vector.bn_stats(out=stats[:, 1], in_=y1)
                    mv = spool.tile([P, 2], F32, tag="mv")
                    nc.vector.bn_aggr(out=mv[:], in_=stats[:])

                    # rstd = 1/sqrt(var+eps): fast-inverse-sqrt seed + 2
                    # Newton iterations, on DVE (cheap small-op overhead)
                    t0 = spool.tile([P, 1], F32, tag="t0")
                    nc.vector.tensor_scalar(t0[:], mv[:, 1:2], LN_EPS, None,
                                            mybir.AluOpType.add)
                    t1 = spool.tile([P, 1], I32, tag="t1")
                    nc.vector.tensor_scalar(t1[:], t0.bitcast(I32)[:], 1, None,
                                            mybir.AluOpType.logical_shift_right)
                    yr = spool.tile([P, 1], F32, tag="yr")
                    nc.vector.tensor_sub(yr.bitcast(I32)[:], magic_t[:], t1[:])
                    for _ in range(2):
                        a = spool.tile([P, 1], F32, tag="nt")
                        nc.vector.tensor_mul(a[:], yr[:], yr[:])
                        nc.vector.tensor_mul(a[:], a[:], t0[:])
                        nc.vector.tensor_scalar(a[:], a[:], -0.5, 1.5,
                                                mybir.AluOpType.mult,
                                                mybir.AluOpType.add)
                        nc.vector.tensor_mul(yr[:], yr[:], a[:])
                    nmr = spool.tile([P, 1], F32, tag="nmr")
                    nc.vector.scalar_tensor_tensor(
                        out=nmr[:], in0=mv[:, 0:1], scalar=-1.0, in1=yr[:],
                        op0=mybir.AluOpType.mult, op1=mybir.AluOpType.mult)

                    # normalize: half0 on ScalarE (Identity: in*rstd + nmr),
                    # half1 on DVE (tensor_scalar) — parallel engine paths.
                    # The very last row-tile does both halves on DVE: the
                    # rsqrt chain lives there, so no cross-engine sem hop
                    # and no queueing behind rt2's Act work in the drain.
                    o = opool.tile([P, D], F32, tag="o")
                    nc.scalar.activation(o[:, 0:NH], y0,
                                         mybir.ActivationFunctionType.Identity,
                                         bias=nmr[:], scale=yr[:])
                    nc.vector.tensor_scalar(o[:, NH:D], y1, yr[:], nmr[:],
                                            mybir.AluOpType.mult,
                                            mybir.AluOpType.add)
                    if not ln_trivial:
                        nc.vector.tensor_mul(o[:], o[:], lngb[:])
                    # residual adds on Pool (keeps DVE free for the next
                    # row-tile's stats/rsqrt chain); the very last row-tile
                    # adds on DVE so the drain doesn't queue behind Pool
                    if blk == NBLK - 1 and r == BR - 1:
                        nc.vector.tensor_add(o[:, 0:NH], o[:, 0:NH], htr[:, r, 0:NH])
                        nc.vector.tensor_add(o[:, NH:D], o[:, NH:D], htr[:, r, NH:D])
                    else:
                        nc.gpsimd.tensor_add(o[:, 0:NH], o[:, 0:NH], htr[:, r, 0:NH])
                        nc.gpsimd.tensor_add(o[:, NH:D], o[:, NH:D], htr[:, r, NH:D])
                    if not ln_trivial:
                        nc.vector.tensor_add(o[:], o[:], lnbb[:])
                    if blk == NBLK - 1:
                        # last block: store halves as they complete
                        nc.sync.dma_start(out=out_d[rows, 0:NH], in_=o[:, 0:NH])
                        nc.sync.dma_start(out=out_d[rows, NH:D], in_=o[:, NH:D])
                    else:
                        nc.sync.dma_start(out=out_d[rows, :], in_=o[:])

            # block-level software pipeline: emit s1(b) before s2(b-1) so
            # the in-order PE stream always has independent matmul work
            # while the Act engine drains a block's silu evictions; loads
            # prefetch one block ahead, w2 after block-1's loads (first
            # needed at s2(b0), which runs after s1(b1))
            tiles = {}

            def alloc_and_load(blk):
                xh = xpool.tile([P, KC, BLK], BF16, name="xh")
                xp = xpool.tile([P, KC, BLK], FP8, name="xp")
                htr = hpool.tile([P, BR, D], BF16, name="htr")
                tiles[blk] = (xh, xp, htr)
                emit_loads(blk, xh, xp, htr)

            def run_stage1(blk):
                g1 = gpool.tile([P, KC, BLK], BF16, name="g1")
                tiles[blk] = (*tiles[blk], g1)
                emit_stage1(blk, tiles[blk][0], tiles[blk][1], g1)

            alloc_and_load(0)
            alloc_and_load(1)
            alloc_and_load(2)
            for k in range(KC):
                nc.sync.dma_start(out=w2[:, k], in_=w2T_v[:, k])
            nc.sync.dma_start(out=tiles[0][2][:], in_=htR_v[0])
            run_stage1(0)
            for blk in range(1, NBLK):
                if blk + 2 < NBLK:
                    alloc_and_load(blk + 2)
                run_stage1(blk)
                emit_stage23(blk - 1, tiles[blk - 1][3], tiles[blk - 1][2])
            emit_stage23(NBLK - 1, tiles[NBLK - 1][3], tiles[NBLK - 1][2])

    nc.compile()
    return nc


def prepare(h_t, phi_x, in_proj_w, in_proj_b, out_proj_w, out_proj_b,
            w1, b1, w2, b2, ln_g, ln_b):
    """Host-side folding + build; returns (nc, in_maps)."""
    # ---- host-side weight folding (fp64) ----
    Wv = in_proj_w[2 * D:].astype(np.float64)
    bv = in_proj_b[2 * D:].astype(np.float64)
    Wo = out_proj_w.astype(np.float64)
    W1a = w1[:, :D].astype(np.float64)
    W1b = w1[:, D:].astype(np.float64)
    WoWv = Wo @ Wv
    Bf = W1b @ WoWv
    c = b1.astype(np.float64) + W1b @ (Wo @ bv + out_proj_b.astype(np.float64))

    AT = np.ascontiguousarray(W1a.T * S1).astype(BF)
    BfT = np.ascontiguousarray(Bf.T * S1).astype(E4)
    w2T = np.ascontiguousarray(np.asarray(w2, np.float64).T).astype(BF)
    c_t = np.ascontiguousarray(c.reshape(KC, P).T).astype(np.float32)

    b2_zero = bool(np.all(b2 == 0))
    ln_trivial = bool(np.all(ln_g == 1) and np.all(ln_b == 0))

    nc = _build(b2_zero, ln_trivial)

    h_t = np.asarray(h_t, dtype=np.float32)
    phi_x = np.asarray(phi_x, dtype=np.float32)

    in_maps = []
    for i in range(N_CORES):
        rows = slice(i * RPC, (i + 1) * RPC)
        ht_i = h_t[rows]
        px_i = phi_x[rows]
        m = {
            "htT": np.ascontiguousarray(ht_i.T).astype(BF),
            "pxT": np.ascontiguousarray(px_i.T).astype(E4),
            "ht_row": ht_i.astype(BF),
            "AT": AT,
            "BfT": BfT,
            "w2T": w2T,
            "c_t": c_t,
        }
        if not b2_zero:
            m["b2"] = np.asarray(b2, dtype=np.float32)
        if not ln_trivial:
            m["ln_g"] = np.asarray(ln_g, dtype=np.float32)
            m["ln_b"] = np.asarray(ln_b, dtype=np.float32)
        in_maps.append(m)

    return nc, in_maps


def kernel(**inputs):
    global _last_results
    nc, in_maps = prepare(**inputs)
    res = run_bass_kernel_spmd(nc, in_maps, core_ids=list(range(N_CORES)),
                               trace=TRACE)
    _last_results = res
    return np.concatenate([r["out"] for r in res.results], axis=0)
